# revision 1
# baseline (speedup 1.0000x reference)
"""Trainium2 Bass kernel for DisentangledSelfAttention (DeBERTa-style).

Shapes (hardcoded): B=2, S=2048, D=1024, H=16, Dh=64, MAX_REL=512.

Sharding: 8 cores; core c handles batch b = c//4 and heads h0 = (c%4)*4 .. +4
(tensor-parallel on heads for q/k/v columns and c_proj rows; data-parallel on
batch). Each core returns a partial transposed c_proj output [1024, 2048];
the host sums the 4 partials per batch and transposes.

Math per core (heads are local 0..3):
  qT/kT [256, 2048] = W.T-slice @ hsT (+bias), v [2048, 256] natural.
  scoresT[j,i] = k_h.T q_h + 8*t[clip(i-j+512)] + 8*kp-term, exp'd with
  scale 1/8, then out = (v|1).T @ exp  -> av[65, i], normalized by row 64.
  c2p uses t = qsum @ PTW (Toeplitz via reversed staging + negative-stride
  identity-select matmul); p2c uses per-(head,jc) kp windows produced by
  matmul in anti-diagonal coords, bounced through DRAM, and re-read with a
  skewed (diagonal) DMA access pattern.  PTW[w] = 8*pos_table[clip(2559-w)]
  serves both terms.
"""
import os

os.environ.setdefault("NEURON_RT_RESET_CORES", "1")

import numpy as np

import concourse.bass as bass
import concourse.bacc as bacc
import concourse.mybir as mybir
import concourse.tile as tile
from concourse.bass_utils import run_bass_kernel_spmd
from concourse.masks import make_identity

F32, BF16, F32R = mybir.dt.float32, mybir.dt.bfloat16, mybir.dt.float32r

B, S, D = 2, 2048, 1024
H, Dh, MAX_REL = 16, 64, 512
NCORES = 8
HPC = H // (NCORES // B)   # heads per core = 4
CLOC = HPC * Dh            # local head-dim columns = 256
WW = 2176                  # kp window width per jc chunk
TSW = 3968                 # c2p staging width
PTWN = 4096                # table rows


def build_nc(max_phase=9):
    nc = bacc.Bacc("TRN2", target_bir_lowering=False)
    hsT = nc.dram_tensor("hsT", [D, S], F32, kind="ExternalInput")
    WqT = nc.dram_tensor("WqT", [D, CLOC], F32, kind="ExternalInput")
    WkT = nc.dram_tensor("WkT", [D, CLOC], F32, kind="ExternalInput")
    WvT = nc.dram_tensor("WvT", [D, CLOC], F32, kind="ExternalInput")
    WcT = nc.dram_tensor("WcT", [CLOC, D], F32, kind="ExternalInput")
    bq = nc.dram_tensor("bq", [CLOC], F32, kind="ExternalInput")
    bk = nc.dram_tensor("bk", [CLOC], F32, kind="ExternalInput")
    bv = nc.dram_tensor("bv", [CLOC], F32, kind="ExternalInput")
    PTWT = nc.dram_tensor("PTWT", [Dh, PTWN], F32, kind="ExternalInput")
    outT = nc.dram_tensor("outT", [D, S], F32, kind="ExternalOutput")

    trev_dram = [nc.dram_tensor(f"trev{h}", [PTWN], F32, kind="Internal")
                 for h in range(HPC)]
    kpwin_dram = [nc.dram_tensor(f"kpwin{h}", [16, 128, WW], BF16,
                                 kind="Internal") for h in range(HPC)]

    with tile.TileContext(nc) as tc:
        with (
            tc.tile_pool(name="consts", bufs=1) as consts,
            tc.tile_pool(name="big", bufs=1) as big,
            tc.tile_pool(name="work", bufs=2) as work,
            tc.tile_pool(name="stage", bufs=1) as stage,
            tc.tile_pool(name="hsst", bufs=2) as hsst,
            tc.tile_pool(name="pp", bufs=2, space="PSUM") as pp,
            tc.tile_pool(name="pav", bufs=2, space="PSUM") as pav,
            tc.tile_pool(name="pkp", bufs=2, space="PSUM") as pkp,
            nc.allow_low_precision(reason="f32r operand rounding throughout"),
        ):
            # ---- Phase 0: constants / weights / tables ----
            WqT_sb = consts.tile([128, 8, CLOC], F32R, name="WqT_sb")
            nc.sync.dma_start(
                WqT_sb[:], WqT.rearrange("(c p) m -> p c m", p=128).bitcast(F32R))
            WkT_sb = consts.tile([128, 8, CLOC], F32R, name="WkT_sb")
            nc.sync.dma_start(
                WkT_sb[:], WkT.rearrange("(c p) m -> p c m", p=128).bitcast(F32R))
            WvT_sb = consts.tile([128, 8, CLOC], F32R, name="WvT_sb")
            nc.sync.dma_start(
                WvT_sb[:], WvT.rearrange("(c p) m -> p c m", p=128).bitcast(F32R))
            WcT_sb = consts.tile([128, 2, D], F32R, name="WcT_sb")
            nc.sync.dma_start(
                WcT_sb[:], WcT.rearrange("(c p) e -> p c e", p=128).bitcast(F32R))
            PTWT_sb = consts.tile([128, PTWN], F32R, name="PTWT_sb")
            nc.sync.dma_start(PTWT_sb[0:Dh, :], PTWT[:].bitcast(F32R))
            nc.sync.dma_start(PTWT_sb[Dh:128, :], PTWT[:].bitcast(F32R))
            bq_sb = consts.tile([128, 2], F32, name="bq_sb")
            nc.sync.dma_start(bq_sb[:], bq.rearrange("(h p) -> p h", p=128))
            bk_sb = consts.tile([128, 2], F32, name="bk_sb")
            nc.sync.dma_start(bk_sb[:], bk.rearrange("(h p) -> p h", p=128))
            bv_bc = consts.tile([128, CLOC], F32, name="bv_bc")
            nc.sync.dma_start(bv_bc[:], bv[None, :].to_broadcast((128, CLOC)))

            ident_f = consts.tile([128, 128], F32, name="ident_f")
            make_identity(nc, ident_f[:])
            ident_r = consts.tile([128, 128], F32R, name="ident_r")
            nc.vector.tensor_copy(out=ident_r[:], in_=ident_f[:])
            ident_b = consts.tile([128, 128], BF16, name="ident_b")
            nc.vector.tensor_copy(out=ident_b[:], in_=ident_f[:])
            ones_f = consts.tile([128, 1], F32, name="ones_f")
            nc.vector.memset(ones_f[:], 1.0)
            ones_r = consts.tile([128, 1], F32R, name="ones_r")
            nc.vector.tensor_copy(out=ones_r[:], in_=ones_f[:])
            onesrow_f = consts.tile([1, 64], F32, name="onesrow_f")
            nc.vector.memset(onesrow_f[:], 1.0)
            onesrow_r = consts.tile([1, 64], F32R, name="onesrow_r")
            nc.vector.tensor_copy(out=onesrow_r[:], in_=onesrow_f[:])

            # ---- Phase 1: projections, streaming hsT in 256-col chunks ----
            qT_sb = big.tile([128, 2, S], F32R, name="qT_sb")
            kT_sb = big.tile([128, 2, S], F32R, name="kT_sb")
            v_sb = big.tile([128, 16, HPC, 65], F32R, name="v_sb")
            hsT_re = hsT.rearrange("(c p) r -> p c r", p=128).bitcast(F32R)
            for rc in range(8):
                r0 = rc * 256
                hs_ck = hsst.tile([128, 8, 256], F32R, name="hs_ck", tag="hsck")
                nc.sync.dma_start(hs_ck[:], hsT_re[:, :, r0:r0 + 256])
                for dst, w_sb, b_sb in ((qT_sb, WqT_sb, bq_sb),
                                        (kT_sb, WkT_sb, bk_sb)):
                    for hh in range(2):
                        ps = pp.tile([128, 512], F32, name="ps_proj", tag="psA")
                        for dc in range(8):
                            nc.tensor.matmul(
                                ps[:, 0:256],
                                w_sb[:, dc, hh * 128:(hh + 1) * 128],
                                hs_ck[:, dc, :],
                                start=(dc == 0), stop=(dc == 7))
                        nc.scalar.activation(
                            out=dst[:, hh, r0:r0 + 256], in_=ps[:, 0:256],
                            func=mybir.ActivationFunctionType.Identity,
                            bias=b_sb[:, hh:hh + 1], scale=1.0)
                for sub in range(2):
                    rr = rc * 2 + sub
                    ps = pp.tile([128, 512], F32, name="ps_v", tag="psA")
                    for dc in range(8):
                        nc.tensor.matmul(
                            ps[:, 0:256], hs_ck[:, dc, sub * 128:(sub + 1) * 128],
                            WvT_sb[:, dc, :], start=(dc == 0), stop=(dc == 7))
                    for h in range(HPC):
                        nc.vector.tensor_tensor(
                            v_sb[:, rr, h, 0:64], ps[:, h * 64:(h + 1) * 64],
                            bv_bc[:, h * 64:(h + 1) * 64], mybir.AluOpType.add)
                        nc.vector.tensor_copy(out=v_sb[:, rr, h, 64:65],
                                              in_=ones_r[:])

            # phase gating for bisection
            PH15 = HPC if max_phase >= 2 else 0
            PH2 = HPC if max_phase >= 3 else 0
            PH3 = HPC if max_phase >= 4 else 0
            PH4 = 4 if max_phase >= 5 else 0

            # ---- Phase 1.5: qsum and t_rev per head ----
            qsum_sb = consts.tile([128, 2], F32R, name="qsum_sb")
            nc.vector.reduce_sum(qsum_sb[:], qT_sb[:], axis=mybir.AxisListType.X)
            for h in range(PH15):
                p0 = (h % 2) * 64
                for yc in range(8):
                    ps = pp.tile([128, 512], F32, name="ps_t", tag="psA")
                    nc.tensor.matmul(
                        ps[0:1, :], qsum_sb[p0:p0 + 64, h // 2:h // 2 + 1],
                        PTWT_sb[p0:p0 + 64, yc * 512:(yc + 1) * 512],
                        start=True, stop=True)
                    tpiece = work.tile([1, 512], F32, name="tpiece")
                    nc.vector.tensor_copy(out=tpiece[:], in_=ps[0:1, :])
                    nc.sync.dma_start(
                        bass.AP(tensor=trev_dram[h], offset=yc * 512,
                                ap=[[512, 1], [1, 512]]), tpiece[0:1, :])

            # ---- Phase 2: kp windows per head -> DRAM ----
            for h in range(PH2):
                p0 = (h % 2) * 64
                for jc in range(16):
                    kpw_sb = work.tile([128, WW], BF16, name="kpw_sb")
                    lhsT = kT_sb[p0:p0 + 64, h // 2, jc * 128:(jc + 1) * 128]
                    for wc in range(5):
                        w0 = wc * 512
                        wid = min(512, WW - w0)
                        ps = pkp.tile([128, 512], F32, name="ps_kp", tag="pskp")
                        nc.tensor.matmul(
                            ps[:, :wid], lhsT,
                            PTWT_sb[p0:p0 + 64, 128 * jc + w0:128 * jc + w0 + wid],
                            start=True, stop=True)
                        nc.vector.tensor_copy(out=kpw_sb[:, w0:w0 + wid],
                                              in_=ps[:, :wid])
                    nc.sync.dma_start(kpwin_dram[h][jc], kpw_sb[:])

            # ---- Phase 3: attention per head ----
            aoT_sb = big.tile([128, 2, S], F32R, name="aoT_sb")
            if max_phase < 5:
                zst = work.tile([128, 512], F32, name="ostage")
                nc.vector.memset(zst[:], 0.0)
                nc.vector.tensor_copy(out=aoT_sb[:, 0, 0:512],
                                      in_=zst[:].bitcast(F32R))
            for h in range(PH3):
                p0 = (h % 2) * 64
                TS2 = stage.tile([128, TSW], F32R, name="TS2")
                nc.sync.dma_start(
                    TS2[:], bass.AP(tensor=trev_dram[h], offset=0,
                                    ap=[[1, 128], [1, TSW]]).bitcast(F32R))
                for istripe in range(4):
                    avp = pav.tile([65, 512], F32, name="avp", tag="pav")
                    for jc in range(16):
                        p2c_nat = work.tile([128, 512], BF16, name="p2c_nat")
                        nc.sync.dma_start(
                            p2c_nat[:],
                            bass.AP(tensor=kpwin_dram[h],
                                    offset=jc * 128 * WW + 512 * istripe,
                                    ap=[[WW + 1, 128], [1, 512]]))
                        sc = pp.tile([128, 512], F32, name="sc", tag="psA")
                        nc.tensor.matmul(
                            sc[:], kT_sb[p0:p0 + 64, h // 2, jc * 128:(jc + 1) * 128],
                            qT_sb[p0:p0 + 64, h // 2, istripe * 512:(istripe + 1) * 512],
                            start=True, stop=False)
                        base = 512 * istripe - 128 * jc + 2048
                        c2p_rhs = bass.AP(
                            tensor=TS2.tensor,
                            offset=TS2.offset + (4095 - base),
                            ap=[[TSW, 128], [-1, 512]])
                        nc.tensor.matmul(sc[:], ident_r[:], c2p_rhs,
                                         start=False, stop=False)
                        nc.tensor.matmul(sc[:], ident_b[:], p2c_nat[:],
                                         start=False, stop=True)
                        sT = work.tile([128, 512], F32R, name="sT")
                        nc.scalar.activation(
                            out=sT[:], in_=sc[:],
                            func=mybir.ActivationFunctionType.Exp, scale=0.125)
                        nc.tensor.matmul(avp[:], v_sb[:, jc, h, :], sT[:],
                                         start=(jc == 0), stop=(jc == 15))
                    av_sb = work.tile([65, 512], F32, name="av_sb")
                    nc.vector.tensor_copy(out=av_sb[:], in_=avp[:])
                    rec = work.tile([1, 512], F32R, name="rec")
                    nc.vector.reciprocal(out=rec[:], in_=av_sb[64:65, :])
                    rbc = pp.tile([128, 512], F32, name="rbc", tag="psA")
                    nc.tensor.matmul(rbc[0:64, :], onesrow_r[:], rec[:],
                                     start=True, stop=True)
                    nc.vector.tensor_tensor(
                        aoT_sb[p0:p0 + 64, h // 2,
                               istripe * 512:(istripe + 1) * 512],
                        av_sb[0:64, :], rbc[0:64, :], mybir.AluOpType.mult)

            # ---- Phase 4: c_proj (transposed partial output) ----
            for rc in range(PH4):
                for ec in range(8):
                    ps = pp.tile([128, 512], F32, name="ps_o", tag="psA")
                    for cc in range(2):
                        nc.tensor.matmul(
                            ps[:], WcT_sb[:, cc, ec * 128:(ec + 1) * 128],
                            aoT_sb[:, cc, rc * 512:(rc + 1) * 512],
                            start=(cc == 0), stop=(cc == 1))
                    ostage = work.tile([128, 512], F32, name="ostage")
                    nc.vector.tensor_copy(out=ostage[:], in_=ps[:])
                    nc.sync.dma_start(
                        outT[ec * 128:(ec + 1) * 128,
                             rc * 512:(rc + 1) * 512], ostage[:])
    nc.compile()
    return nc


_NC_CACHE = None


def _get_nc():
    global _NC_CACHE
    if _NC_CACHE is None:
        _NC_CACHE = build_nc()
    return _NC_CACHE


def _build_in_maps(hidden_states, Wq, bq, Wk, bk, Wv, bv, Wc, pos_table):
    hidden_states = np.asarray(hidden_states, dtype=np.float32)
    Wq, Wk, Wv, Wc = (np.asarray(x, dtype=np.float32) for x in (Wq, Wk, Wv, Wc))
    bq, bk, bv = (np.asarray(x, dtype=np.float32) for x in (bq, bk, bv))
    pos_table = np.asarray(pos_table, dtype=np.float32)

    # PTW[w] = 8 * pos_table[clip(2559 - w, 0, 1023)]  (rows w in [0, 4096))
    w = np.arange(PTWN)
    PTW = 8.0 * pos_table[np.clip(2559 - w, 0, 2 * MAX_REL - 1)]
    PTWT = np.ascontiguousarray(PTW.T)  # [64, 4096]

    hsT = [np.ascontiguousarray(hidden_states[b].T) for b in range(B)]

    in_maps = []
    for c in range(NCORES):
        b = c // (NCORES // B)
        h0 = (c % (NCORES // B)) * HPC
        rows = slice(h0 * Dh, h0 * Dh + CLOC)
        in_maps.append(dict(
            hsT=hsT[b],
            WqT=np.ascontiguousarray(Wq[rows].T),
            WkT=np.ascontiguousarray(Wk[rows].T),
            WvT=np.ascontiguousarray(Wv[rows].T),
            WcT=np.ascontiguousarray(Wc[:, rows].T),
            bq=np.ascontiguousarray(bq[rows]),
            bk=np.ascontiguousarray(bk[rows]),
            bv=np.ascontiguousarray(bv[rows]),
            PTWT=PTWT,
        ))
    return in_maps


def kernel(hidden_states, Wq, bq, Wk, bk, Wv, bv, Wc, pos_table):
    in_maps = _build_in_maps(hidden_states, Wq, bq, Wk, bk, Wv, bv, Wc,
                             pos_table)
    nc = _get_nc()
    results = run_bass_kernel_spmd(nc, in_maps, core_ids=list(range(NCORES)))

    out = np.zeros((B, S, D), dtype=np.float32)
    for c in range(NCORES):
        b = c // (NCORES // B)
        out[b] += results.results[c]["outT"].T
    return out



# revision 2
# speedup vs baseline: 4.4960x; 4.4960x over previous
"""Trainium2 Bass kernel for DisentangledSelfAttention (DeBERTa-style).

Shapes (hardcoded): B=2, S=2048, D=1024, H=16, Dh=64, MAX_REL=512.

Sharding: 8 cores, tensor-parallel on heads only — core c owns global heads
{2c, 2c+1} for BOTH batches (128 q/k/v channels, 128 c_proj rows). Host->device
traffic is minimized: every core receives a distinct 512-token slice of the
(transposed, bf16) hidden states plus only its own 128-channel weight slices;
the full per-batch hidden states are reconstructed on device with an 8-core
AllGather. Each core's c_proj partial [1024, 4096] is summed across cores with
an on-device ReduceScatter, so the cores collectively return exactly the final
output, [128, 4096] bf16 per core.

Math per core (head slots hh = 2*b + l for batch b, local head l):
  qT/kT [128, 2048] per batch = W.T-slice @ hsT (+bias), v natural.
  scoresT[j,i] = k_h.T q_h + 8*t[clip(i-j+512)] + 8*kp-term, exp'd with
  scale 1/8, then out = (v|1).T @ exp -> av[65, i], normalized by row 64.
  c2p uses t = qsum @ PTW (Toeplitz via reversed staging + negative-stride
  identity-select matmul); p2c uses per-(slot,jc) kp windows produced by
  matmul in anti-diagonal coords, bounced through DRAM, and re-read with a
  skewed (diagonal) DMA access pattern.  PTW[w] = 8*pos_table[clip(2559-w)]
  serves both terms.
"""
import os

os.environ.setdefault("NEURON_RT_RESET_CORES", "1")

import numpy as np
import ml_dtypes

import concourse.bass as bass
import concourse.bacc as bacc
import concourse.mybir as mybir
import concourse.tile as tile
from concourse.bass_utils import run_bass_kernel_spmd
from concourse.masks import make_identity

F32, BF16, F32R = mybir.dt.float32, mybir.dt.bfloat16, mybir.dt.float32r
NPBF16 = ml_dtypes.bfloat16

B, S, D = 2, 2048, 1024
H, Dh, MAX_REL = 16, 64, 512
NCORES = 8
CLOC = 128                 # local q/k/v channels = 2 heads * 64
WW = 2176                  # kp window width per jc chunk
TSW = 3968                 # c2p staging width
PTWN = 4096                # table rows
G8 = [[0, 1, 2, 3, 4, 5, 6, 7]]
NSLOT = 4                  # (batch, local head) slots


def build_nc(max_phase=9):
    nc = bacc.Bacc("TRN2", target_bir_lowering=False, num_devices=NCORES)
    hsTp = nc.dram_tensor("hsTp", [D, 512], BF16, kind="ExternalInput")
    WqT = nc.dram_tensor("WqT", [D, CLOC], BF16, kind="ExternalInput")
    WkT = nc.dram_tensor("WkT", [D, CLOC], BF16, kind="ExternalInput")
    WvT = nc.dram_tensor("WvT", [D, CLOC], BF16, kind="ExternalInput")
    WcT = nc.dram_tensor("WcT", [CLOC, D], BF16, kind="ExternalInput")
    bq = nc.dram_tensor("bq", [CLOC], F32, kind="ExternalInput")
    bk = nc.dram_tensor("bk", [CLOC], F32, kind="ExternalInput")
    bv = nc.dram_tensor("bv", [CLOC], F32, kind="ExternalInput")
    PTWT = nc.dram_tensor("PTWT", [Dh, PTWN], BF16, kind="ExternalInput")
    outp = nc.dram_tensor("outp", [CLOC, B * S], BF16, kind="ExternalOutput")

    hsTi = nc.dram_tensor("hsTi", [D, 512], BF16, kind="Internal")
    hsTg = nc.dram_tensor("hsTg", [NCORES * D, 512], BF16, kind="Internal",
                          addr_space="Shared")
    rs_in = nc.dram_tensor("rs_in", [D, B * S], F32, kind="Internal")
    rs_red = nc.dram_tensor("rs_red", [CLOC, B * S], F32, kind="Internal")
    trev_dram = [nc.dram_tensor(f"trev{hh}", [PTWN], F32, kind="Internal")
                 for hh in range(NSLOT)]
    kpwin_dram = [nc.dram_tensor(f"kpwin{hh}", [16, 128, WW], BF16,
                                 kind="Internal") for hh in range(NSLOT)]

    with tile.TileContext(nc) as tc:
        with (
            tc.tile_pool(name="consts", bufs=1) as consts,
            tc.tile_pool(name="big", bufs=1) as big,
            tc.tile_pool(name="work", bufs=2) as work,
            tc.tile_pool(name="stage", bufs=1) as stage,
            tc.tile_pool(name="fin", bufs=1) as fin,
            tc.tile_pool(name="hsst", bufs=2) as hsst,
            tc.tile_pool(name="pp", bufs=2, space="PSUM") as pp,
            tc.tile_pool(name="pav", bufs=2, space="PSUM") as pav,
            tc.tile_pool(name="pkp", bufs=2, space="PSUM") as pkp,
            nc.allow_low_precision(reason="bf16 compute throughout"),
        ):
            # ---- Phase 0a: AllGather hidden states across all 8 cores ----
            nc.sync.dma_start(hsTi[:], hsTp[:])
            nc.gpsimd.collective_compute(
                "AllGather", mybir.AluOpType.bypass, replica_groups=G8,
                ins=[hsTi[:].opt()], outs=[hsTg[:].opt()])

            # ---- Phase 0b: constants / weights / tables ----
            WqT_sb = consts.tile([128, 8, CLOC], BF16, name="WqT_sb")
            nc.sync.dma_start(
                WqT_sb[:], WqT.rearrange("(c p) m -> p c m", p=128))
            WkT_sb = consts.tile([128, 8, CLOC], BF16, name="WkT_sb")
            nc.sync.dma_start(
                WkT_sb[:], WkT.rearrange("(c p) m -> p c m", p=128))
            WvT_sb = consts.tile([128, 8, CLOC], BF16, name="WvT_sb")
            nc.sync.dma_start(
                WvT_sb[:], WvT.rearrange("(c p) m -> p c m", p=128))
            WcT_sb = consts.tile([128, D], BF16, name="WcT_sb")
            nc.sync.dma_start(WcT_sb[:], WcT[:])
            PTWT_sb = consts.tile([128, PTWN], BF16, name="PTWT_sb")
            nc.sync.dma_start(PTWT_sb[0:Dh, :], PTWT[:])
            nc.sync.dma_start(PTWT_sb[Dh:128, :], PTWT[:])
            bq_sb = consts.tile([128, 1], F32, name="bq_sb")
            nc.sync.dma_start(bq_sb[:], bq.rearrange("(h p) -> p h", p=128))
            bk_sb = consts.tile([128, 1], F32, name="bk_sb")
            nc.sync.dma_start(bk_sb[:], bk.rearrange("(h p) -> p h", p=128))
            bv_bc = consts.tile([128, CLOC], F32, name="bv_bc")
            nc.sync.dma_start(bv_bc[:], bv[None, :].to_broadcast((128, CLOC)))

            ident_f = consts.tile([128, 128], F32, name="ident_f")
            make_identity(nc, ident_f[:])
            ident_r = consts.tile([128, 128], F32R, name="ident_r")
            nc.vector.tensor_copy(out=ident_r[:], in_=ident_f[:])
            ident_b = consts.tile([128, 128], BF16, name="ident_b")
            nc.vector.tensor_copy(out=ident_b[:], in_=ident_f[:])
            ones_f = consts.tile([128, 1], F32, name="ones_f")
            nc.vector.memset(ones_f[:], 1.0)
            onesrow_f = consts.tile([1, 64], F32, name="onesrow_f")
            nc.vector.memset(onesrow_f[:], 1.0)
            onesrow_r = consts.tile([1, 64], F32R, name="onesrow_r")
            nc.vector.tensor_copy(out=onesrow_r[:], in_=onesrow_f[:])
            qsum_acc = consts.tile([128, B], F32, name="qsum_acc")
            nc.vector.memset(qsum_acc[:], 0.0)
            qsum_bf = consts.tile([128, B], BF16, name="qsum_bf")

            # ---- Phase 1: projections, streaming gathered hsT blocks ----
            # hsTg row g*128+p holds hsT[(g%8)*128+p, token 512*(g//8)+r]
            hsTg_re = hsTg.rearrange("(g p) r -> p g r", p=128)
            qT_sb = big.tile([128, B, S], BF16, name="qT_sb")
            kT_sb = big.tile([128, B, S], BF16, name="kT_sb")
            v_sb = big.tile([128, B, 16, 2, 65], BF16, name="v_sb")
            for b in range(B):
                for rc in range(4):
                    blk = 4 * b + rc
                    r0 = rc * 512
                    hs_ck = hsst.tile([128, 8, 512], BF16, name="hs_ck",
                                      tag="hsck")
                    nc.sync.dma_start(
                        hs_ck[:], hsTg_re[:, 8 * blk:8 * blk + 8, :])
                    for dst, w_sb, b_sb in ((qT_sb, WqT_sb, bq_sb),
                                            (kT_sb, WkT_sb, bk_sb)):
                        ps = pp.tile([128, 512], F32, name="ps_proj",
                                     tag="psA")
                        for dc in range(8):
                            nc.tensor.matmul(
                                ps[:], w_sb[:, dc, :], hs_ck[:, dc, :],
                                start=(dc == 0), stop=(dc == 7))
                        nc.scalar.activation(
                            out=dst[:, b, r0:r0 + 512], in_=ps[:],
                            func=mybir.ActivationFunctionType.Identity,
                            bias=b_sb[:, 0:1], scale=1.0)
                        if dst is qT_sb:
                            qtmp = work.tile([128, 1], F32, name="qtmp")
                            nc.vector.reduce_sum(qtmp[:], ps[:],
                                                 axis=mybir.AxisListType.X)
                            nc.vector.tensor_tensor(
                                qsum_acc[:, b:b + 1], qsum_acc[:, b:b + 1],
                                qtmp[:], mybir.AluOpType.add)
                    for sub in range(4):
                        rr = rc * 4 + sub
                        ps = pp.tile([128, 512], F32, name="ps_v", tag="psA")
                        for dc in range(8):
                            nc.tensor.matmul(
                                ps[:, 0:CLOC],
                                hs_ck[:, dc, sub * 128:(sub + 1) * 128],
                                WvT_sb[:, dc, :], start=(dc == 0),
                                stop=(dc == 7))
                        for l in range(2):
                            nc.vector.tensor_tensor(
                                v_sb[:, b, rr, l, 0:64],
                                ps[:, l * 64:(l + 1) * 64],
                                bv_bc[:, l * 64:(l + 1) * 64],
                                mybir.AluOpType.add)
                            nc.vector.tensor_copy(
                                out=v_sb[:, b, rr, l, 64:65], in_=ones_f[:])

            # phase gating for bisection
            PH15 = NSLOT if max_phase >= 2 else 0
            PH2 = NSLOT if max_phase >= 3 else 0
            PH3 = NSLOT if max_phase >= 4 else 0
            PH4 = B if max_phase >= 5 else 0

            # ---- Phase 1.5: qsum (incl. 2048*bq) and t_rev per slot ----
            # qsum = sum_l q[l] = sum_l (Wq hs + bq) = acc + S*bq
            for b in range(B):
                nc.scalar.activation(
                    out=qsum_bf[:, b:b + 1], in_=bq_sb[:, 0:1],
                    func=mybir.ActivationFunctionType.Identity,
                    bias=qsum_acc[:, b:b + 1], scale=float(S))
            for hh in range(PH15):
                l, b = hh % 2, hh // 2
                p0 = l * 64
                for yc in range(8):
                    ps = pp.tile([128, 512], F32, name="ps_t", tag="psA")
                    nc.tensor.matmul(
                        ps[0:1, :], qsum_bf[p0:p0 + 64, b:b + 1],
                        PTWT_sb[p0:p0 + 64, yc * 512:(yc + 1) * 512],
                        start=True, stop=True)
                    tpiece = work.tile([1, 512], F32, name="tpiece")
                    nc.vector.tensor_copy(out=tpiece[:], in_=ps[0:1, :])
                    nc.sync.dma_start(
                        bass.AP(tensor=trev_dram[hh], offset=yc * 512,
                                ap=[[512, 1], [1, 512]]), tpiece[0:1, :])

            # ---- Phase 2: kp windows per slot -> DRAM ----
            for hh in range(PH2):
                l, b = hh % 2, hh // 2
                p0 = l * 64
                for jc in range(16):
                    kpw_sb = work.tile([128, WW], BF16, name="kpw_sb")
                    lhsT = kT_sb[p0:p0 + 64, b, jc * 128:(jc + 1) * 128]
                    for wc in range(5):
                        w0 = wc * 512
                        wid = min(512, WW - w0)
                        ps = pkp.tile([128, 512], F32, name="ps_kp",
                                      tag="pskp")
                        nc.tensor.matmul(
                            ps[:, :wid], lhsT,
                            PTWT_sb[p0:p0 + 64,
                                    128 * jc + w0:128 * jc + w0 + wid],
                            start=True, stop=True)
                        nc.vector.tensor_copy(out=kpw_sb[:, w0:w0 + wid],
                                              in_=ps[:, :wid])
                    nc.sync.dma_start(kpwin_dram[hh][jc], kpw_sb[:])

            # ---- Phase 3: attention per slot ----
            aoT_sb = big.tile([128, B, S], BF16, name="aoT_sb")
            if max_phase < 5:
                zst = work.tile([128, 512], F32, name="ostage")
                nc.vector.memset(zst[:], 0.0)
                nc.vector.tensor_copy(out=aoT_sb[:, 0, 0:512],
                                      in_=zst[:])
            for hh in range(PH3):
                l, b = hh % 2, hh // 2
                p0 = l * 64
                TS2 = stage.tile([128, TSW], F32R, name="TS2")
                nc.sync.dma_start(
                    TS2[:], bass.AP(tensor=trev_dram[hh], offset=0,
                                    ap=[[1, 128], [1, TSW]]).bitcast(F32R))
                for istripe in range(4):
                    avp = pav.tile([65, 512], F32, name="avp", tag="pav")
                    for jc in range(16):
                        p2c_nat = work.tile([128, 512], BF16, name="p2c_nat")
                        nc.sync.dma_start(
                            p2c_nat[:],
                            bass.AP(tensor=kpwin_dram[hh],
                                    offset=jc * 128 * WW + 512 * istripe,
                                    ap=[[WW + 1, 128], [1, 512]]))
                        sc = pp.tile([128, 512], F32, name="sc", tag="psA")
                        nc.tensor.matmul(
                            sc[:], kT_sb[p0:p0 + 64, b, jc * 128:(jc + 1) * 128],
                            qT_sb[p0:p0 + 64, b, istripe * 512:(istripe + 1) * 512],
                            start=True, stop=False)
                        base = 512 * istripe - 128 * jc + 2048
                        c2p_rhs = bass.AP(
                            tensor=TS2.tensor,
                            offset=TS2.offset + (4095 - base),
                            ap=[[TSW, 128], [-1, 512]])
                        nc.tensor.matmul(sc[:], ident_r[:], c2p_rhs,
                                         start=False, stop=False)
                        nc.tensor.matmul(sc[:], ident_b[:], p2c_nat[:],
                                         start=False, stop=True)
                        sT = work.tile([128, 512], BF16, name="sT")
                        nc.scalar.activation(
                            out=sT[:], in_=sc[:],
                            func=mybir.ActivationFunctionType.Exp, scale=0.125)
                        nc.tensor.matmul(avp[:], v_sb[:, b, jc, l, :], sT[:],
                                         start=(jc == 0), stop=(jc == 15))
                    av_sb = work.tile([65, 512], F32, name="av_sb")
                    nc.vector.tensor_copy(out=av_sb[:], in_=avp[:])
                    rec = work.tile([1, 512], F32R, name="rec")
                    nc.vector.reciprocal(out=rec[:], in_=av_sb[64:65, :])
                    rbc = pp.tile([128, 512], F32, name="rbc", tag="psA")
                    nc.tensor.matmul(rbc[0:64, :], onesrow_r[:], rec[:],
                                     start=True, stop=True)
                    nc.vector.tensor_tensor(
                        aoT_sb[p0:p0 + 64, b,
                               istripe * 512:(istripe + 1) * 512],
                        av_sb[0:64, :], rbc[0:64, :], mybir.AluOpType.mult)

            # ---- Phase 4: c_proj partials -> rs_in, ReduceScatter, out ----
            for b in range(PH4):
                for rc in range(4):
                    for ec in range(8):
                        ps = pp.tile([128, 512], F32, name="ps_o", tag="psA")
                        nc.tensor.matmul(
                            ps[:], WcT_sb[:, ec * 128:(ec + 1) * 128],
                            aoT_sb[:, b, rc * 512:(rc + 1) * 512],
                            start=True, stop=True)
                        ostage = work.tile([128, 512], F32, name="ostage")
                        nc.vector.tensor_copy(out=ostage[:], in_=ps[:])
                        nc.sync.dma_start(
                            rs_in[ec * 128:(ec + 1) * 128,
                                  b * S + rc * 512:b * S + (rc + 1) * 512],
                            ostage[:])
            nc.gpsimd.collective_compute(
                "ReduceScatter", mybir.AluOpType.add, replica_groups=G8,
                ins=[rs_in[:].opt()], outs=[rs_red[:].opt()])
            red_sb = fin.tile([128, B * S], F32, name="red_sb")
            nc.sync.dma_start(red_sb[:], rs_red[:])
            red_bf = fin.tile([128, B * S], BF16, name="red_bf")
            nc.vector.tensor_copy(out=red_bf[:], in_=red_sb[:])
            nc.sync.dma_start(outp[:], red_bf[:])
    nc.compile()
    return nc


_NC_CACHE = None


def _get_nc():
    global _NC_CACHE
    if _NC_CACHE is None:
        _NC_CACHE = build_nc()
    return _NC_CACHE


def _build_in_maps(hidden_states, Wq, bq, Wk, bk, Wv, bv, Wc, pos_table):
    hidden_states = np.asarray(hidden_states, dtype=np.float32)
    Wq, Wk, Wv, Wc = (np.asarray(x, dtype=np.float32) for x in (Wq, Wk, Wv, Wc))
    bq, bk, bv = (np.asarray(x, dtype=np.float32) for x in (bq, bk, bv))
    pos_table = np.asarray(pos_table, dtype=np.float32)

    # PTW[w] = 8 * pos_table[clip(2559 - w, 0, 1023)]  (rows w in [0, 4096))
    w = np.arange(PTWN)
    PTW = 8.0 * pos_table[np.clip(2559 - w, 0, 2 * MAX_REL - 1)]
    PTWT = np.ascontiguousarray(PTW.T.astype(NPBF16))  # [64, 4096]

    # concat both batches' transposed hidden states: [1024, 4096] bf16
    hsT_all = np.concatenate(
        [hidden_states[b].T for b in range(B)], axis=1).astype(NPBF16)
    WqT_f = Wq.T.astype(NPBF16)
    WkT_f = Wk.T.astype(NPBF16)
    WvT_f = Wv.T.astype(NPBF16)
    WcT_f = Wc.T.astype(NPBF16)

    in_maps = []
    for c in range(NCORES):
        rows = slice(c * CLOC, (c + 1) * CLOC)
        in_maps.append(dict(
            hsTp=np.ascontiguousarray(hsT_all[:, c * 512:(c + 1) * 512]),
            WqT=np.ascontiguousarray(WqT_f[:, rows]),
            WkT=np.ascontiguousarray(WkT_f[:, rows]),
            WvT=np.ascontiguousarray(WvT_f[:, rows]),
            WcT=np.ascontiguousarray(WcT_f[rows, :]),
            bq=np.ascontiguousarray(bq[rows]),
            bk=np.ascontiguousarray(bk[rows]),
            bv=np.ascontiguousarray(bv[rows]),
            PTWT=PTWT,
        ))
    return in_maps


def kernel(hidden_states, Wq, bq, Wk, bk, Wv, bv, Wc, pos_table):
    in_maps = _build_in_maps(hidden_states, Wq, bq, Wk, bk, Wv, bv, Wc,
                             pos_table)
    nc = _get_nc()
    results = run_bass_kernel_spmd(nc, in_maps, core_ids=list(range(NCORES)))

    outT = np.concatenate(
        [np.asarray(results.results[c]["outp"]) for c in range(NCORES)],
        axis=0).astype(np.float32)  # [1024, 4096]
    out = np.empty((B, S, D), dtype=np.float32)
    for b in range(B):
        out[b] = outT[:, b * S:(b + 1) * S].T
    return out


# revision 3
# speedup vs baseline: 4.8457x; 1.0778x over previous
"""Trainium2 Bass kernel for DisentangledSelfAttention (DeBERTa-style).

Shapes (hardcoded): B=2, S=2048, D=1024, H=16, Dh=64, MAX_REL=512.

Sharding: 8 cores, tensor-parallel on heads only — core c owns global heads
{2c, 2c+1} for BOTH batches (128 q/k/v channels, 128 c_proj rows). Host->device
traffic is minimized: each core receives ONE packed bf16 blob holding a
distinct 512-token slice of the (transposed) hidden states, a distinct 512-col
slice of the PTW table, and only its own 128-channel weight slices + biases.
The hidden-state and PTW slices are reconstructed on device with a single
8-core AllGather. Each core's c_proj partial [1024, 4096] is summed across
cores with an on-device ReduceScatter, so the cores collectively return
exactly the final output, [128, 4096] bf16 per core.

Math per core (head slots hh = 2*b + l for batch b, local head l):
  qT/kT [128, 2048] per batch = W.T-slice @ hsT (+bias), v natural.
  scoresT[j,i] = k_h.T q_h + 8*t[clip(i-j+512)] + 8*kp-term, exp'd with
  scale 1/8, then out = (v|1).T @ exp -> av[65, i], normalized by row 64.
  c2p uses t = qsum @ PTW (Toeplitz via reversed staging + negative-stride
  identity-select matmul); p2c uses per-(slot,jc) kp windows produced by
  matmul in anti-diagonal coords, bounced through DRAM, and re-read with a
  skewed (diagonal) DMA access pattern.  PTW[w] = 8*pos_table[clip(2559-w)]
  serves both terms.
"""
import os

os.environ.setdefault("NEURON_RT_RESET_CORES", "1")

import numpy as np
import ml_dtypes

import concourse.bass as bass
import concourse.bacc as bacc
import concourse.mybir as mybir
import concourse.tile as tile
from concourse.bass_utils import run_bass_kernel_spmd
from concourse.masks import make_identity

F32, BF16, F32R = mybir.dt.float32, mybir.dt.bfloat16, mybir.dt.float32r
NPBF16 = ml_dtypes.bfloat16

B, S, D = 2, 2048, 1024
H, Dh, MAX_REL = 16, 64, 512
NCORES = 8
CLOC = 128                 # local q/k/v channels = 2 heads * 64
WW = 2176                  # kp window width per jc chunk
TSW = 3968                 # c2p staging width
PTWN = 4096                # table rows
G8 = [[0, 1, 2, 3, 4, 5, 6, 7]]
NSLOT = 4                  # (batch, local head) slots

# packed input blob layout (element offsets, bf16)
OFF_HS = 0                       # hsT slice [1024, 512]
OFF_PT = OFF_HS + D * 512        # PTWT slice [64, 512]
AG_LEN = OFF_PT + Dh * 512       # AllGather covers [OFF_HS, AG_LEN)
OFF_WQ = AG_LEN                  # WqT slice [1024, 128]
OFF_WK = OFF_WQ + D * CLOC
OFF_WV = OFF_WK + D * CLOC
OFF_WC = OFF_WV + D * CLOC       # WcT slice [128, 1024]
OFF_B = OFF_WC + CLOC * D        # bq|bk|bv slices, [128] each
BLOB_LEN = OFF_B + 3 * CLOC


def build_nc(max_phase=9):
    nc = bacc.Bacc("TRN2", target_bir_lowering=False, num_devices=NCORES)
    blob = nc.dram_tensor("blob", [BLOB_LEN], BF16, kind="ExternalInput")
    outp = nc.dram_tensor("outp", [CLOC, B * S], BF16, kind="ExternalOutput")

    agin = nc.dram_tensor("agin", [AG_LEN], BF16, kind="Internal")
    agg = nc.dram_tensor("agg", [NCORES * AG_LEN], BF16, kind="Internal",
                         addr_space="Shared")
    rs_in = nc.dram_tensor("rs_in", [D, B * S], F32, kind="Internal")
    rs_red = nc.dram_tensor("rs_red", [CLOC, B * S], F32, kind="Internal")
    trev_dram = [nc.dram_tensor(f"trev{hh}", [PTWN], F32, kind="Internal")
                 for hh in range(NSLOT)]
    kpwin_dram = [nc.dram_tensor(f"kpwin{hh}", [16, 128, WW], BF16,
                                 kind="Internal") for hh in range(NSLOT)]

    with tile.TileContext(nc) as tc:
        with (
            tc.tile_pool(name="consts", bufs=1) as consts,
            tc.tile_pool(name="big", bufs=1) as big,
            tc.tile_pool(name="work", bufs=2) as work,
            tc.tile_pool(name="stage", bufs=1) as stage,
            tc.tile_pool(name="fin", bufs=1) as fin,
            tc.tile_pool(name="hsst", bufs=2) as hsst,
            tc.tile_pool(name="pp", bufs=2, space="PSUM") as pp,
            tc.tile_pool(name="pav", bufs=2, space="PSUM") as pav,
            tc.tile_pool(name="pkp", bufs=2, space="PSUM") as pkp,
            nc.allow_low_precision(reason="bf16 compute throughout"),
        ):
            # ---- Phase 0a: AllGather hidden states + PTW table slices ----
            nc.sync.dma_start(agin[:], blob[OFF_HS:AG_LEN])
            nc.gpsimd.collective_compute(
                "AllGather", mybir.AluOpType.bypass, replica_groups=G8,
                ins=[agin[:].opt()], outs=[agg[:].opt()])

            # ---- Phase 0b: constants / weights / biases ----
            WqT_sb = consts.tile([128, 8, CLOC], BF16, name="WqT_sb")
            nc.sync.dma_start(WqT_sb[:], bass.AP(
                tensor=blob, offset=OFF_WQ,
                ap=[[CLOC, 128], [128 * CLOC, 8], [1, CLOC]]))
            WkT_sb = consts.tile([128, 8, CLOC], BF16, name="WkT_sb")
            nc.sync.dma_start(WkT_sb[:], bass.AP(
                tensor=blob, offset=OFF_WK,
                ap=[[CLOC, 128], [128 * CLOC, 8], [1, CLOC]]))
            WvT_sb = consts.tile([128, 8, CLOC], BF16, name="WvT_sb")
            nc.sync.dma_start(WvT_sb[:], bass.AP(
                tensor=blob, offset=OFF_WV,
                ap=[[CLOC, 128], [128 * CLOC, 8], [1, CLOC]]))
            WcT_sb = consts.tile([128, D], BF16, name="WcT_sb")
            nc.sync.dma_start(WcT_sb[:], bass.AP(
                tensor=blob, offset=OFF_WC, ap=[[D, 128], [1, D]]))

            bq_bb = consts.tile([128, 1], BF16, name="bq_bb")
            nc.sync.dma_start(bq_bb[:], bass.AP(
                tensor=blob, offset=OFF_B, ap=[[1, 128], [1, 1]]))
            bq_sb = consts.tile([128, 1], F32, name="bq_sb")
            nc.vector.tensor_copy(out=bq_sb[:], in_=bq_bb[:])
            bk_bb = consts.tile([128, 1], BF16, name="bk_bb")
            nc.sync.dma_start(bk_bb[:], bass.AP(
                tensor=blob, offset=OFF_B + CLOC, ap=[[1, 128], [1, 1]]))
            bk_sb = consts.tile([128, 1], F32, name="bk_sb")
            nc.vector.tensor_copy(out=bk_sb[:], in_=bk_bb[:])
            bv_bb = consts.tile([128, CLOC], BF16, name="bv_bb")
            nc.sync.dma_start(bv_bb[:], bass.AP(
                tensor=blob, offset=OFF_B + 2 * CLOC,
                ap=[[0, 128], [1, CLOC]]))
            bv_bc = consts.tile([128, CLOC], F32, name="bv_bc")
            nc.vector.tensor_copy(out=bv_bc[:], in_=bv_bb[:])

            # PTWT from the gathered table slices (rank r holds cols 512r..)
            PTWT_sb = consts.tile([128, PTWN], BF16, name="PTWT_sb")
            for r in range(8):
                src = bass.AP(tensor=agg, offset=r * AG_LEN + OFF_PT,
                              ap=[[512, Dh], [1, 512]])
                nc.sync.dma_start(PTWT_sb[0:Dh, r * 512:(r + 1) * 512], src)
                nc.sync.dma_start(PTWT_sb[Dh:128, r * 512:(r + 1) * 512], src)

            ident_f = consts.tile([128, 128], F32, name="ident_f")
            make_identity(nc, ident_f[:])
            ident_r = consts.tile([128, 128], F32R, name="ident_r")
            nc.vector.tensor_copy(out=ident_r[:], in_=ident_f[:])
            ident_b = consts.tile([128, 128], BF16, name="ident_b")
            nc.vector.tensor_copy(out=ident_b[:], in_=ident_f[:])
            ones_f = consts.tile([128, 1], F32, name="ones_f")
            nc.vector.memset(ones_f[:], 1.0)
            onesrow_f = consts.tile([1, 64], F32, name="onesrow_f")
            nc.vector.memset(onesrow_f[:], 1.0)
            onesrow_r = consts.tile([1, 64], F32R, name="onesrow_r")
            nc.vector.tensor_copy(out=onesrow_r[:], in_=onesrow_f[:])
            qsum_acc = consts.tile([128, B], F32, name="qsum_acc")
            nc.vector.memset(qsum_acc[:], 0.0)
            qsum_bf = consts.tile([128, B], BF16, name="qsum_bf")

            # ---- Phase 1: projections, streaming gathered hsT blocks ----
            # rank blk's hs slice: element (d, r) at blk*AG_LEN + d*512 + r
            qT_sb = big.tile([128, B, S], BF16, name="qT_sb")
            kT_sb = big.tile([128, B, S], BF16, name="kT_sb")
            v_sb = big.tile([128, B, 16, 2, 65], BF16, name="v_sb")
            for b in range(B):
                for rc in range(4):
                    blk = 4 * b + rc
                    r0 = rc * 512
                    hs_ck = hsst.tile([128, 8, 512], BF16, name="hs_ck",
                                      tag="hsck")
                    nc.sync.dma_start(hs_ck[:], bass.AP(
                        tensor=agg, offset=blk * AG_LEN + OFF_HS,
                        ap=[[512, 128], [128 * 512, 8], [1, 512]]))
                    for dst, w_sb, b_sb in ((qT_sb, WqT_sb, bq_sb),
                                            (kT_sb, WkT_sb, bk_sb)):
                        ps = pp.tile([128, 512], F32, name="ps_proj",
                                     tag="psA")
                        for dc in range(8):
                            nc.tensor.matmul(
                                ps[:], w_sb[:, dc, :], hs_ck[:, dc, :],
                                start=(dc == 0), stop=(dc == 7))
                        nc.scalar.activation(
                            out=dst[:, b, r0:r0 + 512], in_=ps[:],
                            func=mybir.ActivationFunctionType.Identity,
                            bias=b_sb[:, 0:1], scale=1.0)
                        if dst is qT_sb:
                            qtmp = work.tile([128, 1], F32, name="qtmp")
                            nc.vector.reduce_sum(qtmp[:], ps[:],
                                                 axis=mybir.AxisListType.X)
                            nc.vector.tensor_tensor(
                                qsum_acc[:, b:b + 1], qsum_acc[:, b:b + 1],
                                qtmp[:], mybir.AluOpType.add)
                    for sub in range(4):
                        rr = rc * 4 + sub
                        ps = pp.tile([128, 512], F32, name="ps_v", tag="psA")
                        for dc in range(8):
                            nc.tensor.matmul(
                                ps[:, 0:CLOC],
                                hs_ck[:, dc, sub * 128:(sub + 1) * 128],
                                WvT_sb[:, dc, :], start=(dc == 0),
                                stop=(dc == 7))
                        for l in range(2):
                            nc.vector.tensor_tensor(
                                v_sb[:, b, rr, l, 0:64],
                                ps[:, l * 64:(l + 1) * 64],
                                bv_bc[:, l * 64:(l + 1) * 64],
                                mybir.AluOpType.add)
                            nc.vector.tensor_copy(
                                out=v_sb[:, b, rr, l, 64:65], in_=ones_f[:])

            # phase gating for bisection
            PH15 = NSLOT if max_phase >= 2 else 0
            PH2 = NSLOT if max_phase >= 3 else 0
            PH3 = NSLOT if max_phase >= 4 else 0
            PH4 = B if max_phase >= 5 else 0

            # ---- Phase 1.5: qsum (incl. S*bq) and t_rev per slot ----
            for b in range(B):
                nc.scalar.activation(
                    out=qsum_bf[:, b:b + 1], in_=bq_sb[:, 0:1],
                    func=mybir.ActivationFunctionType.Identity,
                    bias=qsum_acc[:, b:b + 1], scale=float(S))
            for hh in range(PH15):
                l, b = hh % 2, hh // 2
                p0 = l * 64
                for yc in range(8):
                    ps = pp.tile([128, 512], F32, name="ps_t", tag="psA")
                    nc.tensor.matmul(
                        ps[0:1, :], qsum_bf[p0:p0 + 64, b:b + 1],
                        PTWT_sb[p0:p0 + 64, yc * 512:(yc + 1) * 512],
                        start=True, stop=True)
                    tpiece = work.tile([1, 512], F32, name="tpiece")
                    nc.vector.tensor_copy(out=tpiece[:], in_=ps[0:1, :])
                    nc.sync.dma_start(
                        bass.AP(tensor=trev_dram[hh], offset=yc * 512,
                                ap=[[512, 1], [1, 512]]), tpiece[0:1, :])

            # ---- Phase 2: kp windows per slot -> DRAM ----
            for hh in range(PH2):
                l, b = hh % 2, hh // 2
                p0 = l * 64
                for jc in range(16):
                    kpw_sb = work.tile([128, WW], BF16, name="kpw_sb")
                    lhsT = kT_sb[p0:p0 + 64, b, jc * 128:(jc + 1) * 128]
                    for wc in range(5):
                        w0 = wc * 512
                        wid = min(512, WW - w0)
                        ps = pkp.tile([128, 512], F32, name="ps_kp",
                                      tag="pskp")
                        nc.tensor.matmul(
                            ps[:, :wid], lhsT,
                            PTWT_sb[p0:p0 + 64,
                                    128 * jc + w0:128 * jc + w0 + wid],
                            start=True, stop=True)
                        nc.vector.tensor_copy(out=kpw_sb[:, w0:w0 + wid],
                                              in_=ps[:, :wid])
                    nc.sync.dma_start(kpwin_dram[hh][jc], kpw_sb[:])

            # ---- Phase 3: attention per slot ----
            aoT_sb = big.tile([128, B, S], BF16, name="aoT_sb")
            if max_phase < 5:
                zst = work.tile([128, 512], F32, name="ostage")
                nc.vector.memset(zst[:], 0.0)
                nc.vector.tensor_copy(out=aoT_sb[:, 0, 0:512], in_=zst[:])
            for hh in range(PH3):
                l, b = hh % 2, hh // 2
                p0 = l * 64
                TS2 = stage.tile([128, TSW], F32R, name="TS2")
                nc.sync.dma_start(
                    TS2[:], bass.AP(tensor=trev_dram[hh], offset=0,
                                    ap=[[1, 128], [1, TSW]]).bitcast(F32R))
                for istripe in range(4):
                    avp = pav.tile([65, 512], F32, name="avp", tag="pav")
                    for jc in range(16):
                        p2c_nat = work.tile([128, 512], BF16, name="p2c_nat")
                        nc.sync.dma_start(
                            p2c_nat[:],
                            bass.AP(tensor=kpwin_dram[hh],
                                    offset=jc * 128 * WW + 512 * istripe,
                                    ap=[[WW + 1, 128], [1, 512]]))
                        sc = pp.tile([128, 512], F32, name="sc", tag="psA")
                        nc.tensor.matmul(
                            sc[:], kT_sb[p0:p0 + 64, b, jc * 128:(jc + 1) * 128],
                            qT_sb[p0:p0 + 64, b, istripe * 512:(istripe + 1) * 512],
                            start=True, stop=False)
                        base = 512 * istripe - 128 * jc + 2048
                        c2p_rhs = bass.AP(
                            tensor=TS2.tensor,
                            offset=TS2.offset + (4095 - base),
                            ap=[[TSW, 128], [-1, 512]])
                        nc.tensor.matmul(sc[:], ident_r[:], c2p_rhs,
                                         start=False, stop=False)
                        nc.tensor.matmul(sc[:], ident_b[:], p2c_nat[:],
                                         start=False, stop=True)
                        sT = work.tile([128, 512], BF16, name="sT")
                        nc.scalar.activation(
                            out=sT[:], in_=sc[:],
                            func=mybir.ActivationFunctionType.Exp, scale=0.125)
                        nc.tensor.matmul(avp[:], v_sb[:, b, jc, l, :], sT[:],
                                         start=(jc == 0), stop=(jc == 15))
                    av_sb = work.tile([65, 512], F32, name="av_sb")
                    nc.vector.tensor_copy(out=av_sb[:], in_=avp[:])
                    rec = work.tile([1, 512], F32R, name="rec")
                    nc.vector.reciprocal(out=rec[:], in_=av_sb[64:65, :])
                    rbc = pp.tile([128, 512], F32, name="rbc", tag="psA")
                    nc.tensor.matmul(rbc[0:64, :], onesrow_r[:], rec[:],
                                     start=True, stop=True)
                    nc.vector.tensor_tensor(
                        aoT_sb[p0:p0 + 64, b,
                               istripe * 512:(istripe + 1) * 512],
                        av_sb[0:64, :], rbc[0:64, :], mybir.AluOpType.mult)

            # ---- Phase 4: c_proj partials -> rs_in, ReduceScatter, out ----
            for b in range(PH4):
                for rc in range(4):
                    for ec in range(8):
                        ps = pp.tile([128, 512], F32, name="ps_o", tag="psA")
                        nc.tensor.matmul(
                            ps[:], WcT_sb[:, ec * 128:(ec + 1) * 128],
                            aoT_sb[:, b, rc * 512:(rc + 1) * 512],
                            start=True, stop=True)
                        ostage = work.tile([128, 512], F32, name="ostage")
                        nc.vector.tensor_copy(out=ostage[:], in_=ps[:])
                        nc.sync.dma_start(
                            rs_in[ec * 128:(ec + 1) * 128,
                                  b * S + rc * 512:b * S + (rc + 1) * 512],
                            ostage[:])
            nc.gpsimd.collective_compute(
                "ReduceScatter", mybir.AluOpType.add, replica_groups=G8,
                ins=[rs_in[:].opt()], outs=[rs_red[:].opt()])
            red_sb = fin.tile([128, B * S], F32, name="red_sb")
            nc.sync.dma_start(red_sb[:], rs_red[:])
            red_bf = fin.tile([128, B * S], BF16, name="red_bf")
            nc.vector.tensor_copy(out=red_bf[:], in_=red_sb[:])
            nc.sync.dma_start(outp[:], red_bf[:])
    nc.compile()
    return nc


_NC_CACHE = None


def _get_nc():
    global _NC_CACHE
    if _NC_CACHE is None:
        _NC_CACHE = build_nc()
    return _NC_CACHE


def _build_in_maps(hidden_states, Wq, bq, Wk, bk, Wv, bv, Wc, pos_table):
    hidden_states = np.asarray(hidden_states, dtype=np.float32)
    Wq, Wk, Wv, Wc = (np.asarray(x, dtype=np.float32) for x in (Wq, Wk, Wv, Wc))
    bq, bk, bv = (np.asarray(x, dtype=np.float32) for x in (bq, bk, bv))
    pos_table = np.asarray(pos_table, dtype=np.float32)

    # PTW[w] = 8 * pos_table[clip(2559 - w, 0, 1023)]  (rows w in [0, 4096))
    w = np.arange(PTWN)
    PTW = 8.0 * pos_table[np.clip(2559 - w, 0, 2 * MAX_REL - 1)]
    PTWT_bf = PTW.T.astype(NPBF16)  # [64, 4096]

    # concat both batches' transposed hidden states: [1024, 4096] bf16
    hsT_all = np.concatenate(
        [hidden_states[b].T for b in range(B)], axis=1).astype(NPBF16)
    WqT_f = Wq.T.astype(NPBF16)
    WkT_f = Wk.T.astype(NPBF16)
    WvT_f = Wv.T.astype(NPBF16)
    WcT_f = Wc.T.astype(NPBF16)
    bq_bf, bk_bf, bv_bf = (x.astype(NPBF16) for x in (bq, bk, bv))

    in_maps = []
    for c in range(NCORES):
        rows = slice(c * CLOC, (c + 1) * CLOC)
        blob = np.empty(BLOB_LEN, NPBF16)
        blob[OFF_HS:OFF_PT].reshape(D, 512)[:] = \
            hsT_all[:, c * 512:(c + 1) * 512]
        blob[OFF_PT:AG_LEN].reshape(Dh, 512)[:] = \
            PTWT_bf[:, c * 512:(c + 1) * 512]
        blob[OFF_WQ:OFF_WK].reshape(D, CLOC)[:] = WqT_f[:, rows]
        blob[OFF_WK:OFF_WV].reshape(D, CLOC)[:] = WkT_f[:, rows]
        blob[OFF_WV:OFF_WC].reshape(D, CLOC)[:] = WvT_f[:, rows]
        blob[OFF_WC:OFF_B].reshape(CLOC, D)[:] = WcT_f[rows, :]
        blob[OFF_B:OFF_B + CLOC] = bq_bf[rows]
        blob[OFF_B + CLOC:OFF_B + 2 * CLOC] = bk_bf[rows]
        blob[OFF_B + 2 * CLOC:OFF_B + 3 * CLOC] = bv_bf[rows]
        in_maps.append(dict(blob=blob))
    return in_maps


def kernel(hidden_states, Wq, bq, Wk, bk, Wv, bv, Wc, pos_table):
    in_maps = _build_in_maps(hidden_states, Wq, bq, Wk, bk, Wv, bv, Wc,
                             pos_table)
    nc = _get_nc()
    results = run_bass_kernel_spmd(nc, in_maps, core_ids=list(range(NCORES)))

    outT = np.concatenate(
        [np.asarray(results.results[c]["outp"]) for c in range(NCORES)],
        axis=0).astype(np.float32)  # [1024, 4096]
    out = np.empty((B, S, D), dtype=np.float32)
    for b in range(B):
        out[b] = outT[:, b * S:(b + 1) * S].T
    return out
